# revision 3
# baseline (speedup 1.0000x reference)
"""Trainium2 Bass kernel for nn_DiarizationLoss (PIT diarization loss).

Strategy (8 NeuronCores, T-sharded data-parallel):
  - Shard T=65536 into 8 slices of TLOC=8192; every core processes all B=32
    samples for its T-slice. Perfectly balanced, one SPMD program.
  - Rewrite the masked pairwise BCE cost + VAD BCE as pure dot products
    over t, computed as ONE packed TensorEngine contraction per core:
      rows (lhsT, bf16):  [lp_0..3, lq_0..3, lpv, lqv]   (Ln via ACT engine)
      cols (rhs,  bf16):  [mt_0..3, mask, vmask]          (DVE fused compare*mul)
    where lp=ln(p+eps), lq=ln((1+eps)-p), mt=labels*mask, vmask=vad*mask,
    mask[t] = (t < len_b) built on-device from an iota table and per-core
    thresholds.  8 samples are packed per matmul (lhsT [128,80] x rhs [128,48])
    and 64 chunks PSUM-accumulate, so the PE does all heavy reduction work.
  - Host combines the tiny per-core partial-sum blocks: PIT permutation min
    over the 4x4 cost matrices, means, and the VAD quotient.

Layout per sample on a core: t_loc = 64*p + q  (p partition, q in [0,64)).
LHS tile c-major per sample: column c occupies [s*640 + c*64, +64) so the
packed matmul AP is a single free dim [[64, 80]] offset q (HW requirement:
stationary matmul AP must have exactly one free dimension).
"""

import warnings

warnings.filterwarnings("ignore")

from contextlib import ExitStack
from itertools import permutations

import numpy as np

import concourse.bass as bass
import concourse.mybir as mybir
import concourse.tile as tile
from concourse import bacc
from concourse.bass_utils import run_bass_kernel_spmd

F32 = mybir.dt.float32
BF16 = mybir.dt.bfloat16
Ln = mybir.ActivationFunctionType.Ln
Alu = mybir.AluOpType

# problem constants (hardcoded per contract)
B, T, S = 32, 65536, 4
EPS = 1e-7
PIT_W, VAD_W = 1.0, 0.5
NCORES = 8
TLOC = T // NCORES          # 8192 timesteps per core
P = 128                     # partitions
Q = TLOC // P               # 64 free chunks per sample
GROUP = 8                   # samples packed per matmul
NG = B // GROUP             # 4 matmul groups
PERMS = np.array(list(permutations(range(S))), dtype=np.int64)  # [24, 4]

_CACHE = {}


def _build_nc(reps=1, loop_n=1):
    nc = bacc.Bacc("TRN2", target_bir_lowering=False, debug=False)

    ps_d = nc.dram_tensor("ps", [B, TLOC, S], F32, kind="ExternalInput")
    lb_d = nc.dram_tensor("lb", [B, TLOC, S], F32, kind="ExternalInput")
    pv_d = nc.dram_tensor("pv", [B, TLOC], F32, kind="ExternalInput")
    vd_d = nc.dram_tensor("vd", [B, TLOC], F32, kind="ExternalInput")
    io1_d = nc.dram_tensor("io1", [P, Q], F32, kind="ExternalInput")
    io4_d = nc.dram_tensor("io4", [P, S * Q], F32, kind="ExternalInput")
    thr_d = nc.dram_tensor("thr", [P, B + 2], F32, kind="ExternalInput")
    out_d = nc.dram_tensor("out", [NG, GROUP * 10, GROUP * 6], F32,
                           kind="ExternalOutput")

    with tile.TileContext(nc) as tc, ExitStack() as ctx:
        const_pool = ctx.enter_context(tc.tile_pool(name="const", bufs=1))
        stage_pool = ctx.enter_context(tc.tile_pool(name="stage", bufs=3))
        lhs_pool = ctx.enter_context(tc.tile_pool(name="lhs", bufs=1))
        rhs_pool = ctx.enter_context(tc.tile_pool(name="rhs", bufs=1))
        psum_pool = ctx.enter_context(
            tc.tile_pool(name="psum", bufs=1, space="PSUM"))
        out_pool = ctx.enter_context(tc.tile_pool(name="outp", bufs=1))

        io1_t = const_pool.tile([P, Q], F32, tag="io1")
        io4_t = const_pool.tile([P, S * Q], F32, tag="io4")
        thr_t = const_pool.tile([P, B + 2], F32, tag="thr")
        nc.sync.dma_start(io1_t[:], io1_d[:])
        nc.sync.dma_start(io4_t[:], io4_d[:])
        nc.sync.dma_start(thr_t[:], thr_d[:])
        eps_ap = thr_t[:, B:B + 1]
        onep_ap = thr_t[:, B + 1:B + 2]

        lhs_ts, rhs_ts = [], []
        for g in range(NG):
            lhs_t = lhs_pool.tile([P, GROUP * Q * 10], BF16, tag=f"lhs{g}")
            rhs_t = rhs_pool.tile([P, GROUP * Q * 6], BF16, tag=f"rhs{g}")
            lhs_ts.append(lhs_t)
            rhs_ts.append(rhs_t)

        io4_v = io4_t[:].rearrange("p (c q) -> p c q", c=S, q=Q)

      # repeated passes (reps/loop_n > 1 only for timing in test.py)
        if loop_n > 1:
            with tc.For_i(0, loop_n, 1):
                for _rep in range(reps):
                    _build_pass(nc, tc, stage_pool, lhs_ts, rhs_ts, psum_pool,
                                out_pool, ps_d, lb_d, pv_d, vd_d, out_d,
                                io1_t, io4_v, thr_t, eps_ap, onep_ap)
        else:
            for _rep in range(reps):
                _build_pass(nc, tc, stage_pool, lhs_ts, rhs_ts, psum_pool,
                            out_pool, ps_d, lb_d, pv_d, vd_d, out_d,
                            io1_t, io4_v, thr_t, eps_ap, onep_ap)

    nc.compile()
    return nc


def _build_pass(nc, tc, stage_pool, lhs_ts, rhs_ts, psum_pool, out_pool,
                ps_d, lb_d, pv_d, vd_d, out_d, io1_t, io4_v, thr_t,
                eps_ap, onep_ap):
        for b in range(B):
            g, s = b // GROUP, b % GROUP
            lhs_r = lhs_ts[g][:].rearrange("p (s c q) -> p s c q",
                                           s=GROUP, c=10, q=Q)
            rhs_r = rhs_ts[g][:].rearrange("p (s c q) -> p s c q",
                                           s=GROUP, c=6, q=Q)

            ps_t = stage_pool.tile([P, Q * S], F32, tag="ps")
            nc.sync.dma_start(ps_t[:],
                              ps_d[b].rearrange("(p q) c -> p (q c)", p=P))
            ps_v = ps_t[:].rearrange("p (q c) -> p c q", q=Q, c=S)
            nc.scalar.activation(lhs_r[:, s, 0:4, :], ps_v, Ln,
                                 bias=eps_ap, scale=1.0)
            nc.scalar.activation(lhs_r[:, s, 4:8, :], ps_v, Ln,
                                 bias=onep_ap, scale=-1.0)

            pv_t = stage_pool.tile([P, Q], F32, tag="pv")
            nc.sync.dma_start(pv_t[:],
                              pv_d[b].rearrange("(p q) -> p q", p=P))
            nc.scalar.activation(lhs_r[:, s, 8, :], pv_t[:], Ln,
                                 bias=eps_ap, scale=1.0)
            nc.scalar.activation(lhs_r[:, s, 9, :], pv_t[:], Ln,
                                 bias=onep_ap, scale=-1.0)

            lb_t = stage_pool.tile([P, Q * S], F32, tag="lb")
            nc.sync.dma_start(lb_t[:],
                              lb_d[b].rearrange("(p q) c -> p (q c)", p=P))
            lb_v = lb_t[:].rearrange("p (q c) -> p c q", q=Q, c=S)
            thr_b = thr_t[:, b:b + 1]
            nc.vector.scalar_tensor_tensor(rhs_r[:, s, 0:4, :], io4_v, thr_b,
                                           lb_v, op0=Alu.is_lt, op1=Alu.mult)
            nc.vector.tensor_scalar(rhs_r[:, s, 4, :], io1_t[:], thr_b, None,
                                    op0=Alu.is_lt)
            vd_t = stage_pool.tile([P, Q], F32, tag="vd")
            nc.sync.dma_start(vd_t[:],
                              vd_d[b].rearrange("(p q) -> p q", p=P))
            nc.vector.scalar_tensor_tensor(rhs_r[:, s, 5, :], io1_t[:], thr_b,
                                           vd_t[:], op0=Alu.is_lt, op1=Alu.mult)

        for g in range(NG):
            lhs_f = lhs_ts[g][:]
            rhs_f = rhs_ts[g][:]
            acc = psum_pool.tile([GROUP * 10, GROUP * 6], F32, tag=f"acc{g}")
            for q in range(Q):
                lhsT = bass.AP(lhs_f.tensor, lhs_f.offset + q,
                               [list(lhs_f.ap[0]), [Q, GROUP * 10]])
                rhs = bass.AP(rhs_f.tensor, rhs_f.offset + q,
                              [list(rhs_f.ap[0]), [Q, GROUP * 6]])
                nc.tensor.matmul(acc[:], lhsT, rhs,
                                 start=(q == 0), stop=(q == Q - 1))
            ot = out_pool.tile([GROUP * 10, GROUP * 6], F32, tag=f"ot{g}")
            nc.vector.tensor_copy(ot[:], acc[:])
            nc.sync.dma_start(out_d[g], ot[:])


def _get_nc(reps=1, loop_n=1):
    key = ("nc", reps, loop_n)
    if key not in _CACHE:
        _CACHE[key] = _build_nc(reps, loop_n)
    return _CACHE[key]


def _make_in_maps(pred_speakers, pred_vad, labels, vad, lengths):
    io1 = (np.arange(P)[:, None] * Q
           + np.arange(Q)[None, :]).astype(np.float32)
    io4 = np.tile(io1, (1, S))
    lens = np.asarray(lengths, dtype=np.float64)
    in_maps = []
    for c in range(NCORES):
        t0 = c * TLOC
        thr = np.zeros((P, B + 2), np.float32)
        thr[:, :B] = (lens - t0).astype(np.float32)[None, :]
        thr[:, B] = EPS
        thr[:, B + 1] = 1.0 + EPS
        in_maps.append({
            "ps": np.ascontiguousarray(pred_speakers[:, t0:t0 + TLOC, :],
                                       dtype=np.float32),
            "lb": np.ascontiguousarray(labels[:, t0:t0 + TLOC, :],
                                       dtype=np.float32),
            "pv": np.ascontiguousarray(pred_vad[:, t0:t0 + TLOC],
                                       dtype=np.float32),
            "vd": np.ascontiguousarray(vad[:, t0:t0 + TLOC],
                                       dtype=np.float32),
            "io1": io1,
            "io4": io4,
            "thr": thr,
        })
    return in_maps


def _combine(outs, lengths):
    """Host reduction of per-core partial-sum blocks -> scalar loss."""
    tot = np.zeros((NG, GROUP * 10, GROUP * 6), np.float64)
    for o in outs:
        tot += o.astype(np.float64)

    lens = np.asarray(lengths, dtype=np.float64)
    speaker_sum = 0.0
    vad_num = 0.0
    for b in range(B):
        g, s = b // GROUP, b % GROUP
        blk = tot[g, 10 * s:10 * s + 10, 6 * s:6 * s + 6]
        P1 = blk[0:4, 0:4]          # sum lp_i * mt_j
        Q1 = blk[4:8, 0:4]          # sum lq_i * mt_j
        Q2 = blk[4:8, 4]            # sum lq_i * mask
        lpv_vm = blk[8, 5]          # sum lpv * vad * mask
        lqv_m = blk[9, 4]           # sum lqv * mask
        lqv_vm = blk[9, 5]          # sum lqv * vad * mask

        term1 = -(P1 - Q1)          # [4,4]
        term2 = -Q2                 # [4]
        msum = lens[b]
        L = (term1 + term2[:, None]) / msum
        perm_losses = L[np.arange(S)[None, :], PERMS].mean(axis=-1)  # [24]
        speaker_sum += perm_losses.min()

        vad_num += -(lpv_vm + lqv_m - lqv_vm)

    speaker_loss = speaker_sum / B
    vad_loss = vad_num / lens.sum()
    return np.float32(PIT_W * speaker_loss + VAD_W * vad_loss)


def kernel(pred_speakers, pred_vad, labels, vad, lengths):
    nc = _get_nc()
    in_maps = _make_in_maps(pred_speakers, pred_vad, labels, vad, lengths)
    res = run_bass_kernel_spmd(nc, in_maps, core_ids=list(range(NCORES)))
    outs = [res.results[c]["out"] for c in range(NCORES)]
    return _combine(outs, lengths)


if __name__ == "__main__":
    rng = np.random.default_rng(0)
    inputs = {
        "pred_speakers": rng.random((B, T, S), np.float32),
        "pred_vad": rng.random((B, T), np.float32),
        "labels": rng.integers(0, 2, (B, T, S)).astype(np.float32),
        "vad": rng.integers(0, 2, (B, T)).astype(np.float32),
        "lengths": np.maximum(rng.integers(0, T, B), T // 2).astype(np.int64),
    }
    print("loss:", kernel(**inputs))


# revision 6
# speedup vs baseline: 1.4158x; 1.4158x over previous
"""Trainium2 Bass kernel for nn_DiarizationLoss (PIT diarization loss).

Strategy (8 NeuronCores, T-sharded data-parallel):
  - Shard T=65536 into 8 slices of TLOC=8192; every core processes all B=32
    samples for its T-slice. Perfectly balanced, one SPMD program.
  - Rewrite the masked pairwise BCE cost + VAD BCE as pure dot products
    over t, computed as ONE packed TensorEngine contraction per core:
      rows (lhsT, bf16):  [lp_0..3, lq_0..3, lpv, lqv]   (Ln via ACT engine)
      cols (rhs,  bf16):  [mt_0..3, mask, vmask]         (DVE compare/mult)
    where lp=ln(p+eps), lq=ln((1+eps)-p), mt=labels*mask, vmask=vad*mask,
    mask[t] = (t < len_b) built on-device from an iota table and per-core
    thresholds.  8 samples are packed per matmul (lhsT [128,80] x rhs
    [128,48]) and 64 chunks PSUM-accumulate, so the PE does all heavy
    reduction work.  All DMA / ACT / DVE work is batched per 8-sample group
    (few large instructions - HWDGE issue overhead and per-op engine
    overheads dominate otherwise).
  - Host combines the tiny per-core partial-sum blocks: PIT permutation min
    over the 4x4 cost matrices, means, and the VAD quotient.

Layout per sample on a core: t_loc = 64*p + q  (p partition, q in [0,64)).
LHS tile c-major per sample: column c occupies [s*640 + c*64, +64) so the
packed matmul AP is a single free dim [[64, 80]] offset q (HW requirement:
the stationary matmul operand AP must have exactly one free dimension).
"""

import warnings

warnings.filterwarnings("ignore")

from contextlib import ExitStack
from itertools import permutations

import numpy as np

import concourse.bass as bass
import concourse.mybir as mybir
import concourse.tile as tile
from concourse import bacc
from concourse.bass_utils import run_bass_kernel_spmd

F32 = mybir.dt.float32
BF16 = mybir.dt.bfloat16
Ln = mybir.ActivationFunctionType.Ln
Alu = mybir.AluOpType

# problem constants (hardcoded per contract)
B, T, S = 32, 65536, 4
EPS = 1e-7
PIT_W, VAD_W = 1.0, 0.5
NCORES = 8
TLOC = T // NCORES          # 8192 timesteps per core
P = 128                     # partitions
Q = TLOC // P               # 64 free chunks per sample
GROUP = 8                   # samples packed per matmul
NG = B // GROUP             # 4 matmul groups
PERMS = np.array(list(permutations(range(S))), dtype=np.int64)  # [24, 4]

_CACHE = {}


def _build_nc(reps=1, loop_n=1):
    nc = bacc.Bacc("TRN2", target_bir_lowering=False, debug=False)

    ps_d = nc.dram_tensor("ps", [B, TLOC, S], F32, kind="ExternalInput")
    lb_d = nc.dram_tensor("lb", [B, TLOC, S], F32, kind="ExternalInput")
    pv_d = nc.dram_tensor("pv", [B, TLOC], F32, kind="ExternalInput")
    vd_d = nc.dram_tensor("vd", [B, TLOC], F32, kind="ExternalInput")
    io1_d = nc.dram_tensor("io1", [P, Q], F32, kind="ExternalInput")
    thr_d = nc.dram_tensor("thr", [P, B + 2], F32, kind="ExternalInput")
    out_d = nc.dram_tensor("out", [NG, GROUP * 10, GROUP * 6], F32,
                           kind="ExternalOutput")

    with tile.TileContext(nc) as tc, ExitStack() as ctx:
        const_pool = ctx.enter_context(tc.tile_pool(name="const", bufs=1))
        stage_pool = ctx.enter_context(tc.tile_pool(name="stage", bufs=2))
        vstage_pool = ctx.enter_context(tc.tile_pool(name="vstage", bufs=1))
        lhs_pool = ctx.enter_context(tc.tile_pool(name="lhs", bufs=1))
        rhs_pool = ctx.enter_context(tc.tile_pool(name="rhs", bufs=1))
        psum_pool = ctx.enter_context(
            tc.tile_pool(name="psum", bufs=1, space="PSUM"))
        out_pool = ctx.enter_context(tc.tile_pool(name="outp", bufs=1))

        io1_t = const_pool.tile([P, Q], F32, tag="io1")
        thr_t = const_pool.tile([P, B + 2], F32, tag="thr")
        nc.sync.dma_start(io1_t[:], io1_d[:])
        nc.sync.dma_start(thr_t[:], thr_d[:])
        eps_ap = thr_t[:, B:B + 1]
        onep_ap = thr_t[:, B + 1:B + 2]

        lhs_ts, rhs_ts = [], []
        for g in range(NG):
            lhs_t = lhs_pool.tile([P, GROUP * Q * 10], BF16, tag=f"lhs{g}")
            rhs_t = rhs_pool.tile([P, GROUP * Q * 6], BF16, tag=f"rhs{g}")
            lhs_ts.append(lhs_t)
            rhs_ts.append(rhs_t)

        def build_pass():
            # all-sample VAD staging + masks
            pv_t = vstage_pool.tile([P, B * Q], F32, tag="pv")
            vd_t = vstage_pool.tile([P, B * Q], F32, tag="vd")
            msk_t = vstage_pool.tile([P, B * Q], F32, tag="msk")
            # pv[b, 64p + q] -> pv_t[p, 64b + q]
            pv_dst = pv_t[:].rearrange("p (b q) -> p b q", b=B, q=Q)
            vd_dst = vd_t[:].rearrange("p (b q) -> p b q", b=B, q=Q)
            nc.sync.dma_start(
                pv_dst, pv_d[:].rearrange("b (p q) -> p b q", p=P))
            nc.sync.dma_start(
                vd_dst, vd_d[:].rearrange("b (p q) -> p b q", p=P))
            # mask32[p, (b q)] = io1[p, q] < thr[p, b]
            nc.vector.tensor_tensor(
                msk_t[:].rearrange("p (b q) -> p b q", b=B, q=Q),
                io1_t[:].unsqueeze(1).broadcast_to([P, B, Q]),
                thr_t[:, :B].unsqueeze(2).broadcast_to([P, B, Q]),
                op=Alu.is_lt)
            msk_r = msk_t[:].rearrange("p (b q) -> p b q", b=B, q=Q)

            for g in range(NG):
                s0 = g * GROUP
                lhs_r = lhs_ts[g][:].rearrange("p (s c q) -> p s c q",
                                               s=GROUP, c=10, q=Q)
                rhs_r = rhs_ts[g][:].rearrange("p (s c q) -> p s c q",
                                               s=GROUP, c=6, q=Q)

                # group staging: [p, s*256] with per-row layout (q, c)
                ps_t = stage_pool.tile([P, GROUP * Q * S], F32, tag="ps")
                nc.sync.dma_start(
                    ps_t[:].rearrange("p (s x) -> p s x", s=GROUP, x=Q * S),
                    ps_d[s0:s0 + GROUP].rearrange("s (p q) c -> p s (q c)",
                                                  p=P))
                ps_v = ps_t[:].rearrange("p (s q c) -> p s c q",
                                         s=GROUP, q=Q, c=S)
                nc.scalar.activation(lhs_r[:, :, 0:4, :], ps_v, Ln,
                                     bias=eps_ap, scale=1.0)
                nc.scalar.activation(lhs_r[:, :, 4:8, :], ps_v, Ln,
                                     bias=onep_ap, scale=-1.0)
                nc.scalar.activation(
                    lhs_r[:, :, 8, :],
                    pv_t[:].rearrange("p (b q) -> p b q",
                                      b=B, q=Q)[:, s0:s0 + GROUP, :],
                    Ln, bias=eps_ap, scale=1.0)
                nc.scalar.activation(
                    lhs_r[:, :, 9, :],
                    pv_t[:].rearrange("p (b q) -> p b q",
                                      b=B, q=Q)[:, s0:s0 + GROUP, :],
                    Ln, bias=onep_ap, scale=-1.0)

                lb_t = stage_pool.tile([P, GROUP * Q * S], F32, tag="lb")
                nc.sync.dma_start(
                    lb_t[:].rearrange("p (s x) -> p s x", s=GROUP, x=Q * S),
                    lb_d[s0:s0 + GROUP].rearrange("s (p q) c -> p s (q c)",
                                                  p=P))
                lb_v = lb_t[:].rearrange("p (s q c) -> p s c q",
                                         s=GROUP, q=Q, c=S)
                # mt = labels * mask (mask broadcast over c)
                nc.vector.tensor_tensor(
                    rhs_r[:, :, 0:4, :], lb_v,
                    msk_r[:, s0:s0 + GROUP, :].unsqueeze(2)
                         .broadcast_to([P, GROUP, S, Q]),
                    op=Alu.mult)
                # mask -> bf16 rhs column
                nc.vector.tensor_copy(rhs_r[:, :, 4, :],
                                      msk_r[:, s0:s0 + GROUP, :])
                # vmask = vad * mask
                nc.vector.tensor_tensor(
                    rhs_r[:, :, 5, :],
                    vd_t[:].rearrange("p (b q) -> p b q",
                                      b=B, q=Q)[:, s0:s0 + GROUP, :],
                    msk_r[:, s0:s0 + GROUP, :],
                    op=Alu.mult)

            for g in range(NG):
                lhs_f = lhs_ts[g][:]
                rhs_f = rhs_ts[g][:]
                acc = psum_pool.tile([GROUP * 10, GROUP * 6], F32,
                                     tag=f"acc{g}")
                for q in range(Q):
                    lhsT = bass.AP(lhs_f.tensor, lhs_f.offset + q,
                                   [list(lhs_f.ap[0]), [Q, GROUP * 10]])
                    rhs = bass.AP(rhs_f.tensor, rhs_f.offset + q,
                                  [list(rhs_f.ap[0]), [Q, GROUP * 6]])
                    nc.tensor.matmul(acc[:], lhsT, rhs,
                                     start=(q == 0), stop=(q == Q - 1))
                ot = out_pool.tile([GROUP * 10, GROUP * 6], F32, tag=f"ot{g}")
                nc.vector.tensor_copy(ot[:], acc[:])
                nc.sync.dma_start(out_d[g], ot[:])

        # reps/loop_n > 1 only for timing-by-differencing in test.py
        if loop_n > 1:
            with tc.For_i(0, loop_n, 1):
                for _ in range(reps):
                    build_pass()
        else:
            for _ in range(reps):
                build_pass()

    nc.compile()
    return nc


def _get_nc(reps=1, loop_n=1):
    key = ("nc", reps, loop_n)
    if key not in _CACHE:
        _CACHE[key] = _build_nc(reps, loop_n)
    return _CACHE[key]


def _make_in_maps(pred_speakers, pred_vad, labels, vad, lengths):
    io1 = (np.arange(P)[:, None] * Q
           + np.arange(Q)[None, :]).astype(np.float32)
    lens = np.asarray(lengths, dtype=np.float64)
    in_maps = []
    for c in range(NCORES):
        t0 = c * TLOC
        thr = np.zeros((P, B + 2), np.float32)
        thr[:, :B] = (lens - t0).astype(np.float32)[None, :]
        thr[:, B] = EPS
        thr[:, B + 1] = 1.0 + EPS
        in_maps.append({
            "ps": np.ascontiguousarray(pred_speakers[:, t0:t0 + TLOC, :],
                                       dtype=np.float32),
            "lb": np.ascontiguousarray(labels[:, t0:t0 + TLOC, :],
                                       dtype=np.float32),
            "pv": np.ascontiguousarray(pred_vad[:, t0:t0 + TLOC],
                                       dtype=np.float32),
            "vd": np.ascontiguousarray(vad[:, t0:t0 + TLOC],
                                       dtype=np.float32),
            "io1": io1,
            "thr": thr,
        })
    return in_maps


def _combine(outs, lengths):
    """Host reduction of per-core partial-sum blocks -> scalar loss."""
    tot = np.zeros((NG, GROUP * 10, GROUP * 6), np.float64)
    for o in outs:
        tot += o.astype(np.float64)

    lens = np.asarray(lengths, dtype=np.float64)
    speaker_sum = 0.0
    vad_num = 0.0
    for b in range(B):
        g, s = b // GROUP, b % GROUP
        blk = tot[g, 10 * s:10 * s + 10, 6 * s:6 * s + 6]
        P1 = blk[0:4, 0:4]          # sum lp_i * mt_j
        Q1 = blk[4:8, 0:4]          # sum lq_i * mt_j
        Q2 = blk[4:8, 4]            # sum lq_i * mask
        lpv_vm = blk[8, 5]          # sum lpv * vad * mask
        lqv_m = blk[9, 4]           # sum lqv * mask
        lqv_vm = blk[9, 5]          # sum lqv * vad * mask

        term1 = -(P1 - Q1)          # [4,4]
        term2 = -Q2                 # [4]
        msum = lens[b]
        L = (term1 + term2[:, None]) / msum
        perm_losses = L[np.arange(S)[None, :], PERMS].mean(axis=-1)  # [24]
        speaker_sum += perm_losses.min()

        vad_num += -(lpv_vm + lqv_m - lqv_vm)

    speaker_loss = speaker_sum / B
    vad_loss = vad_num / lens.sum()
    return np.float32(PIT_W * speaker_loss + VAD_W * vad_loss)


def kernel(pred_speakers, pred_vad, labels, vad, lengths):
    nc = _get_nc()
    in_maps = _make_in_maps(pred_speakers, pred_vad, labels, vad, lengths)
    res = run_bass_kernel_spmd(nc, in_maps, core_ids=list(range(NCORES)))
    outs = [res.results[c]["out"] for c in range(NCORES)]
    return _combine(outs, lengths)


if __name__ == "__main__":
    rng = np.random.default_rng(0)
    inputs = {
        "pred_speakers": rng.random((B, T, S), np.float32),
        "pred_vad": rng.random((B, T), np.float32),
        "labels": rng.integers(0, 2, (B, T, S)).astype(np.float32),
        "vad": rng.integers(0, 2, (B, T)).astype(np.float32),
        "lengths": np.maximum(rng.integers(0, T, B), T // 2).astype(np.int64),
    }
    print("loss:", kernel(**inputs))


# revision 7
# speedup vs baseline: 1.4961x; 1.0568x over previous
"""Trainium2 Bass kernel for nn_DiarizationLoss (PIT diarization loss).

Strategy (8 NeuronCores, T-sharded data-parallel):
  - Shard T=65536 into 8 slices of TLOC=8192; every core processes all B=32
    samples for its T-slice. Perfectly balanced, one SPMD program.
  - Rewrite the masked pairwise BCE cost + VAD BCE as pure dot products
    over t, computed as ONE packed TensorEngine contraction per core:
      rows (lhsT, bf16):  [lp_0..3, lq_0..3, lpv, lqv]   (Ln via ACT engine)
      cols (rhs,  bf16):  [mt_0..3, mask, vmask]         (DVE compare/mult)
    where lp=ln(p+eps), lq=ln((1+eps)-p), mt=labels*mask, vmask=vad*mask,
    mask[t] = (t < len_b) built on-device from an iota table and per-core
    thresholds.  8 samples are packed per matmul (lhsT [128,80] x rhs
    [128,48]) and 64 chunks PSUM-accumulate, so the PE does all heavy
    reduction work.  All DMA / ACT / DVE work is batched per 8-sample group
    (few large instructions - HWDGE issue overhead and per-op engine
    overheads dominate otherwise).
  - Host combines the tiny per-core partial-sum blocks: PIT permutation min
    over the 4x4 cost matrices, means, and the VAD quotient.

Layout per sample on a core: t_loc = 64*p + q  (p partition, q in [0,64)).
LHS tile c-major per sample: column c occupies [s*640 + c*64, +64) so the
packed matmul AP is a single free dim [[64, 80]] offset q (HW requirement:
the stationary matmul operand AP must have exactly one free dimension).
"""

import warnings

warnings.filterwarnings("ignore")

from contextlib import ExitStack
from itertools import permutations

import numpy as np

import concourse.bass as bass
import concourse.mybir as mybir
import concourse.tile as tile
from concourse import bacc
from concourse.bass_utils import run_bass_kernel_spmd

F32 = mybir.dt.float32
BF16 = mybir.dt.bfloat16
Ln = mybir.ActivationFunctionType.Ln
Alu = mybir.AluOpType

# problem constants (hardcoded per contract)
B, T, S = 32, 65536, 4
EPS = 1e-7
PIT_W, VAD_W = 1.0, 0.5
NCORES = 8
TLOC = T // NCORES          # 8192 timesteps per core
P = 128                     # partitions
Q = TLOC // P               # 64 free chunks per sample
GROUP = 8                   # samples packed per matmul
NG = B // GROUP             # 4 matmul groups
PERMS = np.array(list(permutations(range(S))), dtype=np.int64)  # [24, 4]

_CACHE = {}


def _build_nc(reps=1, loop_n=1):
    nc = bacc.Bacc("TRN2", target_bir_lowering=False, debug=False)

    ps_d = nc.dram_tensor("ps", [B, TLOC, S], F32, kind="ExternalInput")
    lb_d = nc.dram_tensor("lb", [B, TLOC, S], F32, kind="ExternalInput")
    pv_d = nc.dram_tensor("pv", [B, TLOC], F32, kind="ExternalInput")
    vd_d = nc.dram_tensor("vd", [B, TLOC], F32, kind="ExternalInput")
    io1_d = nc.dram_tensor("io1", [P, Q], F32, kind="ExternalInput")
    thr_d = nc.dram_tensor("thr", [P, B + 2], F32, kind="ExternalInput")
    out_d = nc.dram_tensor("out", [NG, GROUP * 10, GROUP * 6], F32,
                           kind="ExternalOutput")

    with tile.TileContext(nc) as tc, ExitStack() as ctx:
        const_pool = ctx.enter_context(tc.tile_pool(name="const", bufs=1))
        stage_pool = ctx.enter_context(tc.tile_pool(name="stage", bufs=4))
        vstage_pool = ctx.enter_context(tc.tile_pool(name="vstage", bufs=1))
        lhs_pool = ctx.enter_context(tc.tile_pool(name="lhs", bufs=1))
        rhs_pool = ctx.enter_context(tc.tile_pool(name="rhs", bufs=1))
        psum_pool = ctx.enter_context(
            tc.tile_pool(name="psum", bufs=1, space="PSUM"))
        out_pool = ctx.enter_context(tc.tile_pool(name="outp", bufs=1))

        io1_t = const_pool.tile([P, Q], F32, tag="io1")
        thr_t = const_pool.tile([P, B + 2], F32, tag="thr")
        nc.sync.dma_start(io1_t[:], io1_d[:])
        nc.sync.dma_start(thr_t[:], thr_d[:])
        eps_ap = thr_t[:, B:B + 1]
        onep_ap = thr_t[:, B + 1:B + 2]

        lhs_ts, rhs_ts = [], []
        for g in range(NG):
            lhs_t = lhs_pool.tile([P, GROUP * Q * 10], BF16, tag=f"lhs{g}")
            rhs_t = rhs_pool.tile([P, GROUP * Q * 6], BF16, tag=f"rhs{g}")
            lhs_ts.append(lhs_t)
            rhs_ts.append(rhs_t)

        def build_pass():
            # all-sample VAD staging + masks
            pv_t = vstage_pool.tile([P, B * Q], F32, tag="pv")
            vd_t = vstage_pool.tile([P, B * Q], F32, tag="vd")
            msk_t = vstage_pool.tile([P, B * Q], F32, tag="msk")
            pv_dst = pv_t[:].rearrange("p (b q) -> p b q", b=B, q=Q)
            vd_dst = vd_t[:].rearrange("p (b q) -> p b q", b=B, q=Q)
            nc.sync.dma_start(
                pv_dst, pv_d[:].rearrange("b (p q) -> p b q", p=P))
            nc.gpsimd.dma_start(
                vd_dst, vd_d[:].rearrange("b (p q) -> p b q", p=P))

            # prefetch every group's speaker data (ps on HWDGE, lb on SWDGE)
            ps_ts, lb_ts = [], []
            for g in range(NG):
                s0 = g * GROUP
                ps_t = stage_pool.tile([P, GROUP * Q * S], F32, tag="ps")
                nc.sync.dma_start(
                    ps_t[:].rearrange("p (s x) -> p s x", s=GROUP, x=Q * S),
                    ps_d[s0:s0 + GROUP].rearrange("s (p q) c -> p s (q c)",
                                                  p=P))
                lb_t = stage_pool.tile([P, GROUP * Q * S], F32, tag="lb")
                nc.gpsimd.dma_start(
                    lb_t[:].rearrange("p (s x) -> p s x", s=GROUP, x=Q * S),
                    lb_d[s0:s0 + GROUP].rearrange("s (p q) c -> p s (q c)",
                                                  p=P))
                ps_ts.append(ps_t)
                lb_ts.append(lb_t)

            # mask32[p, (b q)] = io1[p, q] < thr[p, b]
            nc.vector.tensor_tensor(
                msk_t[:].rearrange("p (b q) -> p b q", b=B, q=Q),
                io1_t[:].unsqueeze(1).broadcast_to([P, B, Q]),
                thr_t[:, :B].unsqueeze(2).broadcast_to([P, B, Q]),
                op=Alu.is_lt)
            msk_r = msk_t[:].rearrange("p (b q) -> p b q", b=B, q=Q)

            ot = out_pool.tile([GROUP * 10, NG * GROUP * 6], F32, tag="ot")
            for g in range(NG):
                s0 = g * GROUP
                lhs_r = lhs_ts[g][:].rearrange("p (s c q) -> p s c q",
                                               s=GROUP, c=10, q=Q)
                rhs_r = rhs_ts[g][:].rearrange("p (s c q) -> p s c q",
                                               s=GROUP, c=6, q=Q)

                ps_v = ps_ts[g][:].rearrange("p (s q c) -> p s c q",
                                             s=GROUP, q=Q, c=S)
                nc.scalar.activation(lhs_r[:, :, 0:4, :], ps_v, Ln,
                                     bias=eps_ap, scale=1.0)
                nc.scalar.activation(lhs_r[:, :, 4:8, :], ps_v, Ln,
                                     bias=onep_ap, scale=-1.0)
                nc.scalar.activation(
                    lhs_r[:, :, 8, :],
                    pv_t[:].rearrange("p (b q) -> p b q",
                                      b=B, q=Q)[:, s0:s0 + GROUP, :],
                    Ln, bias=eps_ap, scale=1.0)
                nc.scalar.activation(
                    lhs_r[:, :, 9, :],
                    pv_t[:].rearrange("p (b q) -> p b q",
                                      b=B, q=Q)[:, s0:s0 + GROUP, :],
                    Ln, bias=onep_ap, scale=-1.0)

                lb_v = lb_ts[g][:].rearrange("p (s q c) -> p s c q",
                                             s=GROUP, q=Q, c=S)
                # mt = labels * mask (mask broadcast over c)
                nc.vector.tensor_tensor(
                    rhs_r[:, :, 0:4, :], lb_v,
                    msk_r[:, s0:s0 + GROUP, :].unsqueeze(2)
                         .broadcast_to([P, GROUP, S, Q]),
                    op=Alu.mult)
                # mask -> bf16 rhs column
                nc.vector.tensor_copy(rhs_r[:, :, 4, :],
                                      msk_r[:, s0:s0 + GROUP, :])
                # vmask = vad * mask
                nc.vector.tensor_tensor(
                    rhs_r[:, :, 5, :],
                    vd_t[:].rearrange("p (b q) -> p b q",
                                      b=B, q=Q)[:, s0:s0 + GROUP, :],
                    msk_r[:, s0:s0 + GROUP, :],
                    op=Alu.mult)

                # matmul chain for this group
                lhs_f = lhs_ts[g][:]
                rhs_f = rhs_ts[g][:]
                acc = psum_pool.tile([GROUP * 10, GROUP * 6], F32,
                                     tag=f"acc{g}")
                for q in range(Q):
                    lhsT = bass.AP(lhs_f.tensor, lhs_f.offset + q,
                                   [list(lhs_f.ap[0]), [Q, GROUP * 10]])
                    rhs = bass.AP(rhs_f.tensor, rhs_f.offset + q,
                                  [list(rhs_f.ap[0]), [Q, GROUP * 6]])
                    nc.tensor.matmul(acc[:], lhsT, rhs,
                                     start=(q == 0), stop=(q == Q - 1))
                nc.vector.tensor_copy(
                    ot[:, g * GROUP * 6:(g + 1) * GROUP * 6], acc[:])

            nc.sync.dma_start(
                out_d[:].rearrange("g m n -> m g n"), ot[:].rearrange(
                    "m (g n) -> m g n", g=NG, n=GROUP * 6))

        # reps/loop_n > 1 only for timing-by-differencing in test.py
        if loop_n > 1:
            with tc.For_i(0, loop_n, 1):
                for _ in range(reps):
                    build_pass()
        else:
            for _ in range(reps):
                build_pass()

    nc.compile()
    return nc


def _get_nc(reps=1, loop_n=1):
    key = ("nc", reps, loop_n)
    if key not in _CACHE:
        _CACHE[key] = _build_nc(reps, loop_n)
    return _CACHE[key]


def _make_in_maps(pred_speakers, pred_vad, labels, vad, lengths):
    io1 = (np.arange(P)[:, None] * Q
           + np.arange(Q)[None, :]).astype(np.float32)
    lens = np.asarray(lengths, dtype=np.float64)
    in_maps = []
    for c in range(NCORES):
        t0 = c * TLOC
        thr = np.zeros((P, B + 2), np.float32)
        thr[:, :B] = (lens - t0).astype(np.float32)[None, :]
        thr[:, B] = EPS
        thr[:, B + 1] = 1.0 + EPS
        in_maps.append({
            "ps": np.ascontiguousarray(pred_speakers[:, t0:t0 + TLOC, :],
                                       dtype=np.float32),
            "lb": np.ascontiguousarray(labels[:, t0:t0 + TLOC, :],
                                       dtype=np.float32),
            "pv": np.ascontiguousarray(pred_vad[:, t0:t0 + TLOC],
                                       dtype=np.float32),
            "vd": np.ascontiguousarray(vad[:, t0:t0 + TLOC],
                                       dtype=np.float32),
            "io1": io1,
            "thr": thr,
        })
    return in_maps


def _combine(outs, lengths):
    """Host reduction of per-core partial-sum blocks -> scalar loss."""
    tot = np.zeros((NG, GROUP * 10, GROUP * 6), np.float64)
    for o in outs:
        tot += o.astype(np.float64)

    lens = np.asarray(lengths, dtype=np.float64)
    speaker_sum = 0.0
    vad_num = 0.0
    for b in range(B):
        g, s = b // GROUP, b % GROUP
        blk = tot[g, 10 * s:10 * s + 10, 6 * s:6 * s + 6]
        P1 = blk[0:4, 0:4]          # sum lp_i * mt_j
        Q1 = blk[4:8, 0:4]          # sum lq_i * mt_j
        Q2 = blk[4:8, 4]            # sum lq_i * mask
        lpv_vm = blk[8, 5]          # sum lpv * vad * mask
        lqv_m = blk[9, 4]           # sum lqv * mask
        lqv_vm = blk[9, 5]          # sum lqv * vad * mask

        term1 = -(P1 - Q1)          # [4,4]
        term2 = -Q2                 # [4]
        msum = lens[b]
        L = (term1 + term2[:, None]) / msum
        perm_losses = L[np.arange(S)[None, :], PERMS].mean(axis=-1)  # [24]
        speaker_sum += perm_losses.min()

        vad_num += -(lpv_vm + lqv_m - lqv_vm)

    speaker_loss = speaker_sum / B
    vad_loss = vad_num / lens.sum()
    return np.float32(PIT_W * speaker_loss + VAD_W * vad_loss)


def kernel(pred_speakers, pred_vad, labels, vad, lengths):
    nc = _get_nc()
    in_maps = _make_in_maps(pred_speakers, pred_vad, labels, vad, lengths)
    res = run_bass_kernel_spmd(nc, in_maps, core_ids=list(range(NCORES)))
    outs = [res.results[c]["out"] for c in range(NCORES)]
    return _combine(outs, lengths)


if __name__ == "__main__":
    rng = np.random.default_rng(0)
    inputs = {
        "pred_speakers": rng.random((B, T, S), np.float32),
        "pred_vad": rng.random((B, T), np.float32),
        "labels": rng.integers(0, 2, (B, T, S)).astype(np.float32),
        "vad": rng.integers(0, 2, (B, T)).astype(np.float32),
        "lengths": np.maximum(rng.integers(0, T, B), T // 2).astype(np.int64),
    }
    print("loss:", kernel(**inputs))
